# revision 3
# baseline (speedup 1.0000x reference)
"""Multi-head self-attention Bass/Tile kernel for Trainium2, SPMD over 8 cores.

Problem: B=2, T=4096, D=768, H=12, HD=64 dense MHSA (full TxT scores,
key-padding mask, softmax, out-proj with bias).

Sharding: core c handles batch b=c//4 and query slice q0=(c%4)*1024 for all
12 heads over the full 4096 keys.  No collectives: each core computes a
disjoint [768, 1024] slice of the (transposed) output; the host gathers.

All matmuls contract over the partition dim, so the dataflow is "transposed"
(features on partitions, tokens free):
  phase A: QKV projection.  Q^T per head [64, 1024] stays in SBUF;
           K^T [768, 4096] and V' [12, 4096, 65] staged via DRAM
           (V' carries a ones column per head -> softmax denominator
           falls out of the AV matmul).
  phase B: per head h, per key-tile kt: S[128k, 1024q] = K_h^T.T @ Q_h^T,
           P = exp(S/8 + maskbias_k) on ACT (mask is a per-partition bias),
           O'[65, 512] += V'_kt.T @ P (PSUM accumulation over 32 key tiles).
           Normalize O = O'[0:64] * bcast(1/O'[64]) (PE broadcast matmul).
  phase C: out^T[768, 1024] = Wp^T.T @ O^T + b, DMA out.
"""

import functools
import numpy as np

import concourse.bass as bass
import concourse.mybir as mybir
import concourse.tile as tile
from concourse import bacc
from concourse.bass2jax import (
    _bass_exec_p,
    install_neuronx_cc_hook,
    partition_id_tensor,
)

F32R = mybir.dt.float32r
F32 = mybir.dt.float32
AF = mybir.ActivationFunctionType

B, T, D = 2, 4096, 768
H, HD = 12, 64
N_CORES = 8
CORES_PER_B = 4
QS = T // CORES_PER_B          # 1024 query tokens per core
NB = 1e9                        # mask bias magnitude
DT = D // 128                   # 6 d-tiles
KT = T // 128                   # 32 key tiles
QC = QS // 512                  # 2 query chunks of 512


def build_program(reps: int = 1):
    nc = bacc.Bacc("TRN2", target_bir_lowering=False, debug=False,
                   num_devices=N_CORES)

    xT = nc.dram_tensor("xT", [D, T], F32R, kind="ExternalInput").ap()
    xTq = nc.dram_tensor("xTq", [D, QS], F32R, kind="ExternalInput").ap()
    wqT = nc.dram_tensor("wqT", [D, D], F32R, kind="ExternalInput").ap()
    wkT = nc.dram_tensor("wkT", [D, D], F32R, kind="ExternalInput").ap()
    wvT = nc.dram_tensor("wvT", [D, D], F32R, kind="ExternalInput").ap()
    wpT = nc.dram_tensor("wpT", [D, D], F32R, kind="ExternalInput").ap()
    bp = nc.dram_tensor("bp", [128, DT], F32, kind="ExternalInput").ap()
    mb = nc.dram_tensor("mb", [128, KT], F32, kind="ExternalInput").ap()
    onesc = nc.dram_tensor("onesc", [128, H], F32R, kind="ExternalInput").ap()
    outT = nc.dram_tensor("outT", [D, QS], F32, kind="ExternalOutput").ap()

    KTd = nc.dram_tensor("KTd", [D, T], F32R).ap()          # K^T staging
    Vp = nc.dram_tensor("Vp", [H, T, HD + 1], F32R).ap()    # V' staging

    with tile.TileContext(nc) as tc, nc.allow_low_precision(
            reason="f32r matmul pipeline"):
        _body(nc, tc, reps, xT, xTq, wqT, wkT, wvT, wpT, bp, mb, onesc,
              outT, KTd, Vp)
    nc.compile()
    return nc


def _body(nc, tc, reps, xT, xTq, wqT, wkT, wvT, wpT, bp, mb, onesc,
          outT, KTd, Vp):
    from contextlib import ExitStack

    with ExitStack() as root:
        const = root.enter_context(tc.tile_pool(name="const", bufs=1))
        mb_sb = const.tile([128, KT], F32, tag="mb")
        nc.sync.dma_start(mb_sb[:], mb[:])
        bp_sb = const.tile([128, DT], F32, tag="bp")
        nc.sync.dma_start(bp_sb[:], bp[:])
        ones64 = const.tile([1, 64], F32, tag="ones64")
        nc.vector.memset(ones64[:], 1.0)

        # long-lived per-head Q^T and O^T
        qt_pool = root.enter_context(tc.tile_pool(name="qt", bufs=1))
        ot_pool = root.enter_context(tc.tile_pool(name="ot", bufs=1))

        def emit_once():
            qts = _phase_a(nc, tc, qt_pool, xT, xTq, wqT, wkT, wvT, onesc,
                           KTd, Vp)
            ots = _phase_b(nc, tc, ot_pool, qts, mb_sb, ones64, KTd, Vp)
            _phase_c(nc, tc, ots, wpT, bp_sb, outT)

        if reps == 1:
            emit_once()
        else:
            with tc.For_i(0, reps, 1):
                emit_once()


def _phase_a(nc, tc, qt_pool, xT, xTq, wqT, wkT, wvT, onesc, KTd, Vp):
    from contextlib import ExitStack

    # --- Q^T projection: per-head tiles [64, QS], SBUF-resident ---
    qts = []
    with ExitStack() as s:
        wq_pool = s.enter_context(tc.tile_pool(name="wq", bufs=1))
        xq_pool = s.enter_context(tc.tile_pool(name="xq", bufs=1))
        qps_pool = s.enter_context(
            tc.tile_pool(name="qps", bufs=2, space="PSUM"))

        wq_sb, xq_sb = [], []
        for d in range(DT):
            w = wq_pool.tile([128, D], F32R, tag=f"wq{d}")
            nc.sync.dma_start(w[:], wqT[d * 128:(d + 1) * 128, :])
            wq_sb.append(w)
            xq = xq_pool.tile([128, QS], F32R, tag=f"xq{d}")
            nc.sync.dma_start(xq[:], xTq[d * 128:(d + 1) * 128, :])
            xq_sb.append(xq)

        for h in range(H):
            qt = qt_pool.tile([64, QS], F32R, tag=f"qt{h}")
            for c in range(QC):
                ps = qps_pool.tile([64, 512], F32, tag="qps")
                for d in range(DT):
                    nc.tensor.matmul(
                        ps[:], wq_sb[d][:, h * 64:(h + 1) * 64],
                        xq_sb[d][:, c * 512:(c + 1) * 512],
                        start=(d == 0), stop=(d == DT - 1))
                nc.vector.tensor_copy(qt[:, c * 512:(c + 1) * 512], ps[:])
            qts.append(qt)

    # --- K^T and V' over the full T, staged to DRAM ---
    with ExitStack() as s:
        wkv_pool = s.enter_context(tc.tile_pool(name="wkv", bufs=1))
        xt_pool = s.enter_context(tc.tile_pool(name="xt", bufs=2))
        stage_pool = s.enter_context(tc.tile_pool(name="stage", bufs=3))
        kps_pool = s.enter_context(
            tc.tile_pool(name="kps", bufs=2, space="PSUM"))
        vps_pool = s.enter_context(
            tc.tile_pool(name="vps", bufs=2, space="PSUM"))

        wk_sb, wv_sb = [], []
        for d in range(DT):
            wk = wkv_pool.tile([128, D], F32R, tag=f"wk{d}")
            nc.sync.dma_start(wk[:], wkT[d * 128:(d + 1) * 128, :])
            wk_sb.append(wk)
            wv = wkv_pool.tile([128, D], F32R, tag=f"wv{d}")
            nc.sync.dma_start(wv[:], wvT[d * 128:(d + 1) * 128, :])
            wv_sb.append(wv)

        for tch in range(T // 512):
            tsl = slice(tch * 512, (tch + 1) * 512)
            xt_sb = []
            for d in range(DT):
                xt_t = xt_pool.tile([128, 512], F32R, tag=f"xt{d}")
                nc.sync.dma_start(xt_t[:], xT[d * 128:(d + 1) * 128, tsl])
                xt_sb.append(xt_t)

            # K^T rows e*128..e*128+128, cols tsl
            for e in range(DT):
                ps = kps_pool.tile([128, 512], F32, tag="kps")
                for d in range(DT):
                    nc.tensor.matmul(
                        ps[:], wk_sb[d][:, e * 128:(e + 1) * 128], xt_sb[d][:],
                        start=(d == 0), stop=(d == DT - 1))
                kst = stage_pool.tile([128, 512], F32R, tag="kst")
                nc.vector.tensor_copy(kst[:], ps[:])
                nc.sync.dma_start(KTd[e * 128:(e + 1) * 128, tsl], kst[:])

            # V natural layout [t, e] + ones col per head
            for tt in range(4):
                t0 = tch * 512 + tt * 128
                ps = vps_pool.tile([128, D], F32, tag="vps")
                for d in range(DT):
                    lhs = xt_sb[d][:, tt * 128:(tt + 1) * 128]
                    nc.tensor.matmul(ps[:, 0:512], lhs, wv_sb[d][:, 0:512],
                                     start=(d == 0), stop=(d == DT - 1),
                                     skip_group_check=True)
                    nc.tensor.matmul(ps[:, 512:768], lhs, wv_sb[d][:, 512:768],
                                     start=(d == 0), stop=(d == DT - 1),
                                     skip_group_check=True)
                vst = stage_pool.tile([128, H * (HD + 1)], F32R, tag="vst")
                vst3 = vst[:].rearrange("p (h s) -> p h s", s=HD + 1)
                nc.vector.tensor_copy(
                    vst3[:, :, 0:HD],
                    ps[:].rearrange("p (h s) -> p h s", s=HD))
                nc.sync.dma_start(vst3[:, :, HD:HD + 1], onesc[:])
                for h in range(H):
                    nc.sync.dma_start(
                        Vp[h, t0:t0 + 128, :],
                        vst[:, h * (HD + 1):(h + 1) * (HD + 1)])
    return qts


def _phase_b(nc, tc, ot_pool, qts, mb_sb, ones64, KTd, Vp):
    from contextlib import ExitStack

    ots = []
    with ExitStack() as s:
        kh_pool = s.enter_context(tc.tile_pool(name="kh", bufs=2))
        vh_pool = s.enter_context(tc.tile_pool(name="vh", bufs=2))
        p_pool = s.enter_context(tc.tile_pool(name="p", bufs=3))
        nrm_pool = s.enter_context(tc.tile_pool(name="nrm", bufs=2))
        sp_pool = s.enter_context(
            tc.tile_pool(name="sp", bufs=2, space="PSUM"))
        op_pool = s.enter_context(
            tc.tile_pool(name="op", bufs=1, space="PSUM"))
        bc_pool = s.enter_context(
            tc.tile_pool(name="bc", bufs=1, space="PSUM"))

        for h in range(H):
            kh = kh_pool.tile([64, T], F32R, tag="kh")
            nc.sync.dma_start(kh[:], KTd[h * 64:(h + 1) * 64, :])
            vh = vh_pool.tile([128, KT * (HD + 1)], F32R, tag="vh")
            nc.sync.dma_start(
                vh[:].rearrange("p (kt s) -> p kt s", s=HD + 1),
                Vp[h].rearrange("(kt p) s -> p kt s", p=128))

            ops = [op_pool.tile([65, 512], F32, tag=f"op{c}", name=f"op{c}")
                   for c in range(QC)]
            for kt in range(KT):
                sp = sp_pool.tile([128, QC * 512], F32, tag="sp")
                for c in range(QC):
                    nc.tensor.matmul(
                        sp[:, c * 512:(c + 1) * 512],
                        kh[:, kt * 128:(kt + 1) * 128],
                        qts[h][:, c * 512:(c + 1) * 512],
                        start=True, stop=True, skip_group_check=True)
                p = p_pool.tile([128, QC * 512], F32R, tag="p")
                nc.scalar.activation(p[:], sp[:], AF.Exp,
                                     bias=mb_sb[:, kt:kt + 1], scale=0.125)
                for c in range(QC):
                    nc.tensor.matmul(
                        ops[c][:],
                        vh[:, kt * (HD + 1):(kt + 1) * (HD + 1)],
                        p[:, c * 512:(c + 1) * 512],
                        start=(kt == 0), stop=(kt == KT - 1))

            ot = ot_pool.tile([64, QS], F32R, tag=f"ot{h}")
            for c in range(QC):
                recip = nrm_pool.tile([1, 512], F32, tag="recip")
                nc.vector.reciprocal(recip[:], ops[c][64:65, :])
                bc = bc_pool.tile([64, 512], F32, tag="bc")
                nc.tensor.matmul(bc[:], ones64[:], recip[:],
                                 start=True, stop=True)
                bc_sb = nrm_pool.tile([64, 512], F32, tag="bc_sb")
                nc.vector.tensor_copy(bc_sb[:], bc[:])
                nc.vector.tensor_mul(ot[:, c * 512:(c + 1) * 512],
                                     ops[c][0:64, :], bc_sb[:])
            ots.append(ot)
    return ots


def _phase_c(nc, tc, ots, wpT, bp_sb, outT):
    from contextlib import ExitStack

    with ExitStack() as s:
        wp_pool = s.enter_context(tc.tile_pool(name="wp", bufs=1))
        ost_pool = s.enter_context(tc.tile_pool(name="ost", bufs=3))
        pps_pool = s.enter_context(
            tc.tile_pool(name="pps", bufs=2, space="PSUM"))

        wp_sb = []
        for h in range(H):
            wp = wp_pool.tile([64, D], F32R, tag=f"wp{h}")
            nc.sync.dma_start(wp[:], wpT[h * 64:(h + 1) * 64, :])
            wp_sb.append(wp)

        for m in range(DT):
            for c in range(QC):
                ps = pps_pool.tile([128, 512], F32, tag="pps")
                for h in range(H):
                    nc.tensor.matmul(
                        ps[:], wp_sb[h][:, m * 128:(m + 1) * 128],
                        ots[h][:, c * 512:(c + 1) * 512],
                        start=(h == 0), stop=(h == H - 1))
                ost = ost_pool.tile([128, 512], F32, tag="ost")
                nc.vector.tensor_scalar_add(ost[:], ps[:], bp_sb[:, m:m + 1])
                nc.sync.dma_start(
                    outT[m * 128:(m + 1) * 128, c * 512:(c + 1) * 512],
                    ost[:])


# ---------------------------------------------------------------- host side

@functools.lru_cache(maxsize=None)
def _get_runner(reps: int = 1):
    import jax
    from jax.sharding import Mesh, PartitionSpec
    from jax.experimental.shard_map import shard_map

    nc = build_program(reps)
    install_neuronx_cc_hook()
    partition_name = (nc.partition_id_tensor.name
                      if nc.partition_id_tensor else None)
    in_names, out_names, out_avals, out_shapes = [], [], [], []
    for alloc in nc.m.functions[0].allocations:
        if not isinstance(alloc, mybir.MemoryLocationSet):
            continue
        name = alloc.memorylocations[0].name
        if alloc.kind == "ExternalInput":
            if name != partition_name:
                in_names.append(name)
        elif alloc.kind == "ExternalOutput":
            out_names.append(name)
            shape = tuple(alloc.tensor_shape)
            dtype = mybir.dt.np(alloc.dtype)
            out_avals.append(jax.core.ShapedArray(shape, dtype))
            out_shapes.append((shape, dtype))
    n_params = len(in_names)
    n_outs = len(out_avals)
    all_in_names = list(in_names) + list(out_names)
    if partition_name is not None:
        all_in_names.append(partition_name)
    donate = tuple(range(n_params, n_params + n_outs))

    def _bodyf(*args):
        operands = list(args)
        if partition_name is not None:
            operands.append(partition_id_tensor())
        outs = _bass_exec_p.bind(
            *operands,
            out_avals=tuple(out_avals),
            in_names=tuple(all_in_names),
            out_names=tuple(out_names),
            lowering_input_output_aliases=(),
            sim_require_finite=True,
            sim_require_nnan=True,
            nc=nc,
        )
        return tuple(outs)

    devices = jax.devices()[:N_CORES]
    mesh = Mesh(np.asarray(devices), ("core",))
    in_specs = (PartitionSpec("core"),) * (n_params + n_outs)
    out_specs = (PartitionSpec("core"),) * len(out_names)
    sharded = jax.jit(
        shard_map(_bodyf, mesh=mesh, in_specs=in_specs, out_specs=out_specs,
                  check_rep=False),
        donate_argnums=donate, keep_unused=True,
    )

    def run(in_maps):
        import jax as _jax
        per_core = [[np.asarray(m[n]) for n in in_names] for m in in_maps]
        concat_in = [np.concatenate([per_core[c][i] for c in range(N_CORES)],
                                    axis=0) for i in range(n_params)]
        concat_zeros = [np.zeros((N_CORES * s[0], *s[1:]), dt)
                        for (s, dt) in out_shapes]
        out_arrs = sharded(*concat_in, *concat_zeros)
        _jax.block_until_ready(out_arrs)
        return [
            {name: np.asarray(out_arrs[i]).reshape(
                N_CORES, *out_shapes[i][0])[c]
             for i, name in enumerate(out_names)}
            for c in range(N_CORES)
        ]

    return run


def make_in_maps(x, mask, w_qkv, w_proj, b_proj):
    x = np.asarray(x, np.float32)
    mask = np.asarray(mask)
    w_qkv = np.asarray(w_qkv, np.float32)
    w_proj = np.asarray(w_proj, np.float32)
    b_proj = np.asarray(b_proj, np.float32)

    wqT = np.ascontiguousarray(w_qkv[0:D].T)
    wkT = np.ascontiguousarray(w_qkv[D:2 * D].T)
    wvT = np.ascontiguousarray(w_qkv[2 * D:3 * D].T)
    wpT = np.ascontiguousarray(w_proj.T)
    bp = np.ascontiguousarray(b_proj.reshape(DT, 128).T)
    onesc = np.ones((128, H), np.float32)

    xTs = [np.ascontiguousarray(x[b].T) for b in range(B)]
    mbs = [np.ascontiguousarray(
        np.where(mask[b], np.float32(-NB), np.float32(0.0))
        .astype(np.float32).reshape(KT, 128).T) for b in range(B)]

    in_maps = []
    for c in range(N_CORES):
        b, qi = divmod(c, CORES_PER_B)
        q0 = qi * QS
        in_maps.append({
            "xT": xTs[b],
            "xTq": np.ascontiguousarray(xTs[b][:, q0:q0 + QS]),
            "wqT": wqT, "wkT": wkT, "wvT": wvT, "wpT": wpT,
            "bp": bp, "mb": mbs[b], "onesc": onesc,
        })
    return in_maps


def assemble_output(results):
    out = np.empty((B, T, D), np.float32)
    for c in range(N_CORES):
        b, qi = divmod(c, CORES_PER_B)
        q0 = qi * QS
        out[b, q0:q0 + QS, :] = results[c]["outT"].T
    return out


def kernel(x, mask, w_qkv, w_proj, b_proj):
    run = _get_runner(1)
    in_maps = make_in_maps(x, mask, w_qkv, w_proj, b_proj)
    results = run(in_maps)
    return assemble_output(results)


# revision 12
# speedup vs baseline: 11.8910x; 11.8910x over previous
"""Multi-head self-attention Bass/Tile kernel for Trainium2, SPMD over 8 cores.

Problem: B=2, T=4096, D=768, H=12, HD=64 dense MHSA (full TxT scores,
key-padding mask, softmax, out-proj with bias).

Sharding: core c handles batch b=c//4 and query slice q0=(c%4)*1024 for all
12 heads over the full 4096 keys.  No collectives: each core computes a
disjoint [768, 1024] slice of the (transposed) output; the host gathers.

All matmuls contract over the partition dim, so the dataflow is "transposed"
(features on partitions, tokens free):
  phase A: QKV projection.  Q^T per head [64, 1024] stays in SBUF;
           K^T [768, 4096] and V' [12, 4096, 65] staged via DRAM
           (V' carries a ones column per head -> softmax denominator
           falls out of the AV matmul).
  phase B: per head h, per key-tile kt: S[128k, 1024q] = K_h^T.T @ Q_h^T,
           P = exp(S/8 + maskbias_k) on ACT (mask is a per-partition bias),
           O'[65, 512] += V'_kt.T @ P (PSUM accumulation over 32 key tiles).
           Normalize O = O'[0:64] * bcast(1/O'[64]) (PE broadcast matmul).
  phase C: out^T[768, 1024] = Wp^T.T @ O^T + b, DMA out.
"""

import functools
import numpy as np

import concourse.bass as bass
import concourse.mybir as mybir
import concourse.tile as tile
from concourse import bacc
from concourse.bass2jax import (
    _bass_exec_p,
    install_neuronx_cc_hook,
    partition_id_tensor,
)

F32R = mybir.dt.float32r
F32 = mybir.dt.float32
AF = mybir.ActivationFunctionType

B, T, D = 2, 4096, 768
H, HD = 12, 64
N_CORES = 8
CORES_PER_B = 4
QS = T // CORES_PER_B          # 1024 query tokens per core
NB = 1e9                        # mask bias magnitude
DT = D // 128                   # 6 d-tiles
KT = T // 128                   # 32 key tiles
QC = QS // 512                  # 2 query chunks of 512


def build_program(reps: int = 1):
    nc = bacc.Bacc("TRN2", target_bir_lowering=False, debug=False,
                   num_devices=N_CORES)

    xT = nc.dram_tensor("xT", [D, T], F32R, kind="ExternalInput").ap()
    xTq = nc.dram_tensor("xTq", [D, QS], F32R, kind="ExternalInput").ap()
    wqT = nc.dram_tensor("wqT", [D, D], F32R, kind="ExternalInput").ap()
    wkT = nc.dram_tensor("wkT", [D, D], F32R, kind="ExternalInput").ap()
    wvT = nc.dram_tensor("wvT", [D, D], F32R, kind="ExternalInput").ap()
    wpT = nc.dram_tensor("wpT", [D, D], F32R, kind="ExternalInput").ap()
    bp = nc.dram_tensor("bp", [128, DT], F32, kind="ExternalInput").ap()
    mb = nc.dram_tensor("mb", [128, KT], F32, kind="ExternalInput").ap()
    onesc = nc.dram_tensor("onesc", [128, H], F32R, kind="ExternalInput").ap()
    outT = nc.dram_tensor("outT", [D, QS], F32, kind="ExternalOutput").ap()

    KTd = nc.dram_tensor("KTd", [D, T], F32R).ap()          # K^T staging
    Vp = nc.dram_tensor("Vp", [T, H * (HD + 1)], F32R).ap()  # V' staging

    with tile.TileContext(nc) as tc, nc.allow_low_precision(
            reason="f32r matmul pipeline"):
        _body(nc, tc, reps, xT, xTq, wqT, wkT, wvT, wpT, bp, mb, onesc,
              outT, KTd, Vp)
    nc.compile()
    return nc


def _body(nc, tc, reps, xT, xTq, wqT, wkT, wvT, wpT, bp, mb, onesc,
          outT, KTd, Vp):
    from contextlib import ExitStack

    with ExitStack() as root:
        const = root.enter_context(tc.tile_pool(name="const", bufs=1))
        mb_sb = const.tile([128, KT], F32, tag="mb")
        nc.sync.dma_start(mb_sb[:], mb[:])
        bp_sb = const.tile([128, DT], F32, tag="bp")
        nc.sync.dma_start(bp_sb[:], bp[:])
        ones64 = const.tile([1, 64], F32, tag="ones64")
        nc.vector.memset(ones64[:], 1.0)
        onesr = const.tile([128, H], F32R, tag="onesr")
        nc.sync.dma_start(onesr[:], onesc[:])

        # long-lived per-head Q^T and O^T
        qt_pool = root.enter_context(tc.tile_pool(name="qt", bufs=1))
        ot_pool = root.enter_context(tc.tile_pool(name="ot", bufs=1))

        def emit_once():
            qts = _phase_a(nc, tc, qt_pool, xT, xTq, wqT, wkT, wvT, onesr,
                           KTd, Vp)
            ots = _phase_b(nc, tc, ot_pool, qts, mb_sb, ones64, KTd, Vp)
            _phase_c(nc, tc, ots, wpT, bp_sb, outT)

        if reps == 1:
            emit_once()
        elif reps < 0:
            for _ in range(-reps):
                emit_once()
        else:
            with tc.For_i(0, reps, 1):
                emit_once()


def _phase_a(nc, tc, qt_pool, xT, xTq, wqT, wkT, wvT, onesr, KTd, Vp):
    from contextlib import ExitStack

    # --- Q^T projection: per-head tiles [64, QS], SBUF-resident ---
    qts = []
    with ExitStack() as s:
        wq_pool = s.enter_context(tc.tile_pool(name="wq", bufs=1))
        xq_pool = s.enter_context(tc.tile_pool(name="xq", bufs=1))
        qps_pool = s.enter_context(
            tc.tile_pool(name="qps", bufs=2, space="PSUM"))

        wq_sb, xq_sb = [], []
        for d in range(DT):
            w = wq_pool.tile([128, D], F32R, tag=f"wq{d}")
            nc.sync.dma_start(w[:], wqT[d * 128:(d + 1) * 128, :])
            wq_sb.append(w)
            xq = xq_pool.tile([128, QS], F32R, tag=f"xq{d}")
            nc.sync.dma_start(xq[:], xTq[d * 128:(d + 1) * 128, :])
            xq_sb.append(xq)

        for h in range(H):
            qt = qt_pool.tile([64, QS], F32R, tag=f"qt{h}")
            for c in range(QC):
                ps = qps_pool.tile([64, 512], F32, tag="qps")
                for d in range(DT):
                    nc.tensor.matmul(
                        ps[:], wq_sb[d][:, h * 64:(h + 1) * 64],
                        xq_sb[d][:, c * 512:(c + 1) * 512],
                        start=(d == 0), stop=(d == DT - 1))
                nc.vector.tensor_copy(qt[:, c * 512:(c + 1) * 512], ps[:])
            qts.append(qt)

    # --- K^T and V' over the full T, staged to DRAM ---
    with ExitStack() as s:
        wkv_pool = s.enter_context(tc.tile_pool(name="wkv", bufs=1))
        xt_pool = s.enter_context(tc.tile_pool(name="xt", bufs=2))
        stage_pool = s.enter_context(tc.tile_pool(name="stage", bufs=3))
        kps_pool = s.enter_context(
            tc.tile_pool(name="kps", bufs=2, space="PSUM"))
        vps_pool = s.enter_context(
            tc.tile_pool(name="vps", bufs=2, space="PSUM"))

        wk_sb, wv_sb = [], []
        for d in range(DT):
            wk = wkv_pool.tile([128, D], F32R, tag=f"wk{d}")
            nc.sync.dma_start(wk[:], wkT[d * 128:(d + 1) * 128, :])
            wk_sb.append(wk)
            wv = wkv_pool.tile([128, D], F32R, tag=f"wv{d}")
            nc.sync.dma_start(wv[:], wvT[d * 128:(d + 1) * 128, :])
            wv_sb.append(wv)

        for tch in range(T // 1024):
            tsl = slice(tch * 1024, (tch + 1) * 1024)
            xt_sb = []
            for d in range(DT):
                xt_t = xt_pool.tile([128, 1024], F32R, tag=f"xt{d}")
                nc.sync.dma_start(xt_t[:], xT[d * 128:(d + 1) * 128, tsl])
                xt_sb.append(xt_t)

            # K^T rows e*128..e*128+128, cols tsl
            for e in range(DT):
                kst = stage_pool.tile([128, 1024], F32R, tag="kst")
                for half in range(2):
                    hs = slice(half * 512, (half + 1) * 512)
                    ps = kps_pool.tile([128, 512], F32, tag="kps")
                    for d in range(DT):
                        nc.tensor.matmul(
                            ps[:], wk_sb[d][:, e * 128:(e + 1) * 128],
                            xt_sb[d][:, hs],
                            start=(d == 0), stop=(d == DT - 1))
                    nc.vector.tensor_copy(kst[:, hs], ps[:])
                nc.sync.dma_start(KTd[e * 128:(e + 1) * 128, tsl], kst[:])

            # V natural layout [t, e] + ones col per head
            for tt in range(8):
                t0 = tch * 1024 + tt * 128
                ps = vps_pool.tile([128, D], F32, tag="vps")
                for d in range(DT):
                    lhs = xt_sb[d][:, tt * 128:(tt + 1) * 128]
                    nc.tensor.matmul(ps[:, 0:512], lhs, wv_sb[d][:, 0:512],
                                     start=(d == 0), stop=(d == DT - 1),
                                     skip_group_check=True)
                    nc.tensor.matmul(ps[:, 512:768], lhs, wv_sb[d][:, 512:768],
                                     start=(d == 0), stop=(d == DT - 1),
                                     skip_group_check=True)
                vst = stage_pool.tile([128, H * (HD + 1)], F32R, tag="vst")
                vst3 = vst[:].rearrange("p (h s) -> p h s", s=HD + 1)
                nc.vector.tensor_copy(
                    vst3[:, :, 0:HD],
                    ps[:].rearrange("p (h s) -> p h s", s=HD))
                nc.vector.tensor_copy(
                    vst3[:, :, HD:HD + 1],
                    onesr[:].rearrange("p (h o) -> p h o", o=1))
                nc.sync.dma_start(Vp[t0:t0 + 128, :], vst[:])
    return qts


def _phase_b(nc, tc, ot_pool, qts, mb_sb, ones64, KTd, Vp):
    from contextlib import ExitStack

    ots = []
    with ExitStack() as s:
        kh_pool = s.enter_context(tc.tile_pool(name="kh", bufs=2))
        vh_pool = s.enter_context(tc.tile_pool(name="vh", bufs=2))
        p_pool = s.enter_context(tc.tile_pool(name="p", bufs=3))
        nrm_pool = s.enter_context(tc.tile_pool(name="nrm", bufs=2))
        sp_pool = s.enter_context(
            tc.tile_pool(name="sp", bufs=2, space="PSUM"))
        op_pool = s.enter_context(
            tc.tile_pool(name="op", bufs=1, space="PSUM"))
        bc_pool = s.enter_context(
            tc.tile_pool(name="bc", bufs=1, space="PSUM"))

        for h in range(H):
            kh = kh_pool.tile([64, T], F32R, tag="kh")
            nc.sync.dma_start(kh[:], KTd[h * 64:(h + 1) * 64, :])
            vh = vh_pool.tile([128, KT * (HD + 1)], F32R, tag="vh")
            nc.sync.dma_start(
                vh[:].rearrange("p (kt s) -> p kt s", s=HD + 1),
                Vp.rearrange("(kt p) (h s) -> p kt h s", p=128,
                             s=HD + 1)[:, :, h, :])

            ops = [op_pool.tile([65, 512], F32, tag=f"op{c}", name=f"op{c}")
                   for c in range(QC)]
            for kt in range(KT):
                sp = sp_pool.tile([128, QC * 512], F32, tag="sp")
                for c in range(QC):
                    nc.tensor.matmul(
                        sp[:, c * 512:(c + 1) * 512],
                        kh[:, kt * 128:(kt + 1) * 128],
                        qts[h][:, c * 512:(c + 1) * 512],
                        start=True, stop=True, skip_group_check=True)
                p = p_pool.tile([128, QC * 512], F32R, tag="p")
                nc.scalar.activation(p[:], sp[:], AF.Exp,
                                     bias=mb_sb[:, kt:kt + 1], scale=0.125)
                for c in range(QC):
                    nc.tensor.matmul(
                        ops[c][:],
                        vh[:, kt * (HD + 1):(kt + 1) * (HD + 1)],
                        p[:, c * 512:(c + 1) * 512],
                        start=(kt == 0), stop=(kt == KT - 1))

            ot = ot_pool.tile([64, QS], F32R, tag=f"ot{h}")
            for c in range(QC):
                recip = nrm_pool.tile([1, 512], F32, tag="recip")
                nc.vector.reciprocal(recip[:], ops[c][64:65, :])
                bc = bc_pool.tile([64, 512], F32, tag="bc")
                nc.tensor.matmul(bc[:], ones64[:], recip[:],
                                 start=True, stop=True)
                bc_sb = nrm_pool.tile([64, 512], F32, tag="bc_sb")
                nc.vector.tensor_copy(bc_sb[:], bc[:])
                nc.vector.tensor_mul(ot[:, c * 512:(c + 1) * 512],
                                     ops[c][0:64, :], bc_sb[:])
            ots.append(ot)
    return ots


def _phase_c(nc, tc, ots, wpT, bp_sb, outT):
    from contextlib import ExitStack

    with ExitStack() as s:
        wp_pool = s.enter_context(tc.tile_pool(name="wp", bufs=1))
        ost_pool = s.enter_context(tc.tile_pool(name="ost", bufs=3))
        pps_pool = s.enter_context(
            tc.tile_pool(name="pps", bufs=2, space="PSUM"))

        wp_sb = []
        for h in range(H):
            wp = wp_pool.tile([64, D], F32R, tag=f"wp{h}")
            nc.sync.dma_start(wp[:], wpT[h * 64:(h + 1) * 64, :])
            wp_sb.append(wp)

        for m in range(DT):
            for c in range(QC):
                ps = pps_pool.tile([128, 512], F32, tag="pps")
                for h in range(H):
                    nc.tensor.matmul(
                        ps[:], wp_sb[h][:, m * 128:(m + 1) * 128],
                        ots[h][:, c * 512:(c + 1) * 512],
                        start=(h == 0), stop=(h == H - 1))
                ost = ost_pool.tile([128, 512], F32, tag="ost")
                nc.vector.tensor_scalar_add(ost[:], ps[:], bp_sb[:, m:m + 1])
                nc.sync.dma_start(
                    outT[m * 128:(m + 1) * 128, c * 512:(c + 1) * 512],
                    ost[:])


# ---------------------------------------------------------------- host side

@functools.lru_cache(maxsize=None)
def _get_runner(reps: int = 1):
    import jax
    from jax.sharding import Mesh, PartitionSpec
    from jax.experimental.shard_map import shard_map

    nc = build_program(reps)
    install_neuronx_cc_hook()
    partition_name = (nc.partition_id_tensor.name
                      if nc.partition_id_tensor else None)
    in_names, out_names, out_avals, out_shapes = [], [], [], []
    for alloc in nc.m.functions[0].allocations:
        if not isinstance(alloc, mybir.MemoryLocationSet):
            continue
        name = alloc.memorylocations[0].name
        if alloc.kind == "ExternalInput":
            if name != partition_name:
                in_names.append(name)
        elif alloc.kind == "ExternalOutput":
            out_names.append(name)
            shape = tuple(alloc.tensor_shape)
            dtype = mybir.dt.np(alloc.dtype)
            out_avals.append(jax.core.ShapedArray(shape, dtype))
            out_shapes.append((shape, dtype))
    n_params = len(in_names)
    n_outs = len(out_avals)
    all_in_names = list(in_names) + list(out_names)
    if partition_name is not None:
        all_in_names.append(partition_name)
    donate = tuple(range(n_params, n_params + n_outs))

    def _bodyf(*args):
        operands = list(args)
        if partition_name is not None:
            operands.append(partition_id_tensor())
        outs = _bass_exec_p.bind(
            *operands,
            out_avals=tuple(out_avals),
            in_names=tuple(all_in_names),
            out_names=tuple(out_names),
            lowering_input_output_aliases=(),
            sim_require_finite=True,
            sim_require_nnan=True,
            nc=nc,
        )
        return tuple(outs)

    devices = jax.devices()[:N_CORES]
    mesh = Mesh(np.asarray(devices), ("core",))
    in_specs = (PartitionSpec("core"),) * (n_params + n_outs)
    out_specs = (PartitionSpec("core"),) * len(out_names)
    sharded = jax.jit(
        shard_map(_bodyf, mesh=mesh, in_specs=in_specs, out_specs=out_specs,
                  check_rep=False),
        donate_argnums=donate, keep_unused=True,
    )

    def run(in_maps):
        import jax as _jax
        per_core = [[np.asarray(m[n]) for n in in_names] for m in in_maps]
        concat_in = [np.concatenate([per_core[c][i] for c in range(N_CORES)],
                                    axis=0) for i in range(n_params)]
        concat_zeros = [np.zeros((N_CORES * s[0], *s[1:]), dt)
                        for (s, dt) in out_shapes]
        out_arrs = sharded(*concat_in, *concat_zeros)
        _jax.block_until_ready(out_arrs)
        return [
            {name: np.asarray(out_arrs[i]).reshape(
                N_CORES, *out_shapes[i][0])[c]
             for i, name in enumerate(out_names)}
            for c in range(N_CORES)
        ]

    return run


def make_in_maps(x, mask, w_qkv, w_proj, b_proj):
    x = np.asarray(x, np.float32)
    mask = np.asarray(mask)
    w_qkv = np.asarray(w_qkv, np.float32)
    w_proj = np.asarray(w_proj, np.float32)
    b_proj = np.asarray(b_proj, np.float32)

    wqT = np.ascontiguousarray(w_qkv[0:D].T)
    wkT = np.ascontiguousarray(w_qkv[D:2 * D].T)
    wvT = np.ascontiguousarray(w_qkv[2 * D:3 * D].T)
    wpT = np.ascontiguousarray(w_proj.T)
    bp = np.ascontiguousarray(b_proj.reshape(DT, 128).T)
    onesc = np.ones((128, H), np.float32)

    xTs = [np.ascontiguousarray(x[b].T) for b in range(B)]
    mbs = [np.ascontiguousarray(
        np.where(mask[b], np.float32(-NB), np.float32(0.0))
        .astype(np.float32).reshape(KT, 128).T) for b in range(B)]

    in_maps = []
    for c in range(N_CORES):
        b, qi = divmod(c, CORES_PER_B)
        q0 = qi * QS
        in_maps.append({
            "xT": xTs[b],
            "xTq": np.ascontiguousarray(xTs[b][:, q0:q0 + QS]),
            "wqT": wqT, "wkT": wkT, "wvT": wvT, "wpT": wpT,
            "bp": bp, "mb": mbs[b], "onesc": onesc,
        })
    return in_maps


def assemble_output(results):
    out = np.empty((B, T, D), np.float32)
    for c in range(N_CORES):
        b, qi = divmod(c, CORES_PER_B)
        q0 = qi * QS
        out[b, q0:q0 + QS, :] = results[c]["outT"].T
    return out


def kernel(x, mask, w_qkv, w_proj, b_proj):
    run = _get_runner(1)
    in_maps = make_in_maps(x, mask, w_qkv, w_proj, b_proj)
    results = run(in_maps)
    return assemble_output(results)


# revision 13
# speedup vs baseline: 12.3035x; 1.0347x over previous
"""Multi-head self-attention Bass/Tile kernel for Trainium2, SPMD over 8 cores.

Problem: B=2, T=4096, D=768, H=12, HD=64 dense MHSA (full TxT scores,
key-padding mask, softmax, out-proj with bias).

Sharding: core c handles batch b=c//4 and query slice q0=(c%4)*1024 for all
12 heads over the full 4096 keys.  No collectives: each core computes a
disjoint [768, 1024] slice of the (transposed) output; the host gathers.

All matmuls contract over the partition dim, so the dataflow is "transposed"
(features on partitions, tokens free):
  phase A: QKV projection.  Q^T per head [64, 1024] stays in SBUF;
           K^T [768, 4096] and V' [12, 4096, 65] staged via DRAM
           (V' carries a ones column per head -> softmax denominator
           falls out of the AV matmul).
  phase B: per head h, per key-tile kt: S[128k, 1024q] = K_h^T.T @ Q_h^T,
           P = exp(S/8 + maskbias_k) on ACT (mask is a per-partition bias),
           O'[65, 512] += V'_kt.T @ P (PSUM accumulation over 32 key tiles).
           Normalize O = O'[0:64] * bcast(1/O'[64]) (PE broadcast matmul).
  phase C: out^T[768, 1024] = Wp^T.T @ O^T + b, DMA out.
"""

import functools
import numpy as np

import concourse.bass as bass
import concourse.mybir as mybir
import concourse.tile as tile
from concourse import bacc
from concourse.bass2jax import (
    _bass_exec_p,
    install_neuronx_cc_hook,
    partition_id_tensor,
)

F32R = mybir.dt.float32r
F32 = mybir.dt.float32
BF16 = mybir.dt.bfloat16
USE_BF16 = True
MMDT = BF16 if USE_BF16 else F32R
AF = mybir.ActivationFunctionType

B, T, D = 2, 4096, 768
H, HD = 12, 64
N_CORES = 8
CORES_PER_B = 4
QS = T // CORES_PER_B          # 1024 query tokens per core
NB = 1e9                        # mask bias magnitude
DT = D // 128                   # 6 d-tiles
KT = T // 128                   # 32 key tiles
QC = QS // 512                  # 2 query chunks of 512


def build_program(reps: int = 1):
    nc = bacc.Bacc("TRN2", target_bir_lowering=False, debug=False,
                   num_devices=N_CORES)

    xT = nc.dram_tensor("xT", [D, T], MMDT, kind="ExternalInput").ap()
    xTq = nc.dram_tensor("xTq", [D, QS], MMDT, kind="ExternalInput").ap()
    wqT = nc.dram_tensor("wqT", [D, D], MMDT, kind="ExternalInput").ap()
    wkT = nc.dram_tensor("wkT", [D, D], MMDT, kind="ExternalInput").ap()
    wvT = nc.dram_tensor("wvT", [D, D], MMDT, kind="ExternalInput").ap()
    wpT = nc.dram_tensor("wpT", [D, D], MMDT, kind="ExternalInput").ap()
    bp = nc.dram_tensor("bp", [128, DT], F32, kind="ExternalInput").ap()
    mb = nc.dram_tensor("mb", [128, KT], F32, kind="ExternalInput").ap()
    onesc = nc.dram_tensor("onesc", [128, H], MMDT, kind="ExternalInput").ap()
    outT = nc.dram_tensor("outT", [D, QS], F32, kind="ExternalOutput").ap()

    KTd = nc.dram_tensor("KTd", [D, T], MMDT).ap()          # K^T staging
    Vp = nc.dram_tensor("Vp", [T, H * (HD + 1)], MMDT).ap()  # V' staging

    with tile.TileContext(nc) as tc, nc.allow_low_precision(
            reason="f32r matmul pipeline"):
        _body(nc, tc, reps, xT, xTq, wqT, wkT, wvT, wpT, bp, mb, onesc,
              outT, KTd, Vp)
    nc.compile()
    return nc


def _body(nc, tc, reps, xT, xTq, wqT, wkT, wvT, wpT, bp, mb, onesc,
          outT, KTd, Vp):
    from contextlib import ExitStack

    with ExitStack() as root:
        const = root.enter_context(tc.tile_pool(name="const", bufs=1))
        mb_sb = const.tile([128, KT], F32, tag="mb")
        nc.sync.dma_start(mb_sb[:], mb[:])
        bp_sb = const.tile([128, DT], F32, tag="bp")
        nc.sync.dma_start(bp_sb[:], bp[:])
        ones64 = const.tile([1, 64], F32, tag="ones64")
        nc.vector.memset(ones64[:], 1.0)
        onesr = const.tile([128, H], MMDT, tag="onesr")
        nc.sync.dma_start(onesr[:], onesc[:])

        # long-lived per-head Q^T and O^T
        qt_pool = root.enter_context(tc.tile_pool(name="qt", bufs=1))
        ot_pool = root.enter_context(tc.tile_pool(name="ot", bufs=1))

        def emit_once():
            qts = _phase_a(nc, tc, qt_pool, xT, xTq, wqT, wkT, wvT, onesr,
                           KTd, Vp)
            ots = _phase_b(nc, tc, ot_pool, qts, mb_sb, ones64, KTd, Vp)
            _phase_c(nc, tc, ots, wpT, bp_sb, outT)

        if reps == 1:
            emit_once()
        elif reps < 0:
            for _ in range(-reps):
                emit_once()
        else:
            with tc.For_i(0, reps, 1):
                emit_once()


def _phase_a(nc, tc, qt_pool, xT, xTq, wqT, wkT, wvT, onesr, KTd, Vp):
    from contextlib import ExitStack

    # --- Q^T projection: per-head tiles [64, QS], SBUF-resident ---
    qts = []
    with ExitStack() as s:
        wq_pool = s.enter_context(tc.tile_pool(name="wq", bufs=1))
        xq_pool = s.enter_context(tc.tile_pool(name="xq", bufs=1))
        qps_pool = s.enter_context(
            tc.tile_pool(name="qps", bufs=2, space="PSUM"))

        wq_sb, xq_sb = [], []
        for d in range(DT):
            w = wq_pool.tile([128, D], MMDT, tag=f"wq{d}")
            nc.sync.dma_start(w[:], wqT[d * 128:(d + 1) * 128, :])
            wq_sb.append(w)
            xq = xq_pool.tile([128, QS], MMDT, tag=f"xq{d}")
            nc.sync.dma_start(xq[:], xTq[d * 128:(d + 1) * 128, :])
            xq_sb.append(xq)

        for h in range(H):
            qt = qt_pool.tile([64, QS], MMDT, tag=f"qt{h}")
            for c in range(QC):
                ps = qps_pool.tile([64, 512], F32, tag="qps")
                for d in range(DT):
                    nc.tensor.matmul(
                        ps[:], wq_sb[d][:, h * 64:(h + 1) * 64],
                        xq_sb[d][:, c * 512:(c + 1) * 512],
                        start=(d == 0), stop=(d == DT - 1))
                nc.vector.tensor_copy(qt[:, c * 512:(c + 1) * 512], ps[:])
            qts.append(qt)

    # --- K^T and V' over the full T, staged to DRAM ---
    with ExitStack() as s:
        wkv_pool = s.enter_context(tc.tile_pool(name="wkv", bufs=1))
        xt_pool = s.enter_context(tc.tile_pool(name="xt", bufs=2))
        stage_pool = s.enter_context(tc.tile_pool(name="stage", bufs=3))
        kps_pool = s.enter_context(
            tc.tile_pool(name="kps", bufs=2, space="PSUM"))
        vps_pool = s.enter_context(
            tc.tile_pool(name="vps", bufs=2, space="PSUM"))

        wk_sb, wv_sb = [], []
        for d in range(DT):
            wk = wkv_pool.tile([128, D], MMDT, tag=f"wk{d}")
            nc.sync.dma_start(wk[:], wkT[d * 128:(d + 1) * 128, :])
            wk_sb.append(wk)
            wv = wkv_pool.tile([128, D], MMDT, tag=f"wv{d}")
            nc.sync.dma_start(wv[:], wvT[d * 128:(d + 1) * 128, :])
            wv_sb.append(wv)

        for tch in range(T // 1024):
            tsl = slice(tch * 1024, (tch + 1) * 1024)
            xt_sb = []
            for d in range(DT):
                xt_t = xt_pool.tile([128, 1024], MMDT, tag=f"xt{d}")
                nc.sync.dma_start(xt_t[:], xT[d * 128:(d + 1) * 128, tsl])
                xt_sb.append(xt_t)

            # K^T rows e*128..e*128+128, cols tsl
            for e in range(DT):
                kst = stage_pool.tile([128, 1024], MMDT, tag="kst")
                for half in range(2):
                    hs = slice(half * 512, (half + 1) * 512)
                    ps = kps_pool.tile([128, 512], F32, tag="kps")
                    for d in range(DT):
                        nc.tensor.matmul(
                            ps[:], wk_sb[d][:, e * 128:(e + 1) * 128],
                            xt_sb[d][:, hs],
                            start=(d == 0), stop=(d == DT - 1))
                    nc.vector.tensor_copy(kst[:, hs], ps[:])
                nc.sync.dma_start(KTd[e * 128:(e + 1) * 128, tsl], kst[:])

            # V natural layout [t, e] + ones col per head
            for tt in range(8):
                t0 = tch * 1024 + tt * 128
                ps = vps_pool.tile([128, D], F32, tag="vps")
                for d in range(DT):
                    lhs = xt_sb[d][:, tt * 128:(tt + 1) * 128]
                    nc.tensor.matmul(ps[:, 0:512], lhs, wv_sb[d][:, 0:512],
                                     start=(d == 0), stop=(d == DT - 1),
                                     skip_group_check=True)
                    nc.tensor.matmul(ps[:, 512:768], lhs, wv_sb[d][:, 512:768],
                                     start=(d == 0), stop=(d == DT - 1),
                                     skip_group_check=True)
                vst = stage_pool.tile([128, H * (HD + 1)], MMDT, tag="vst")
                vst3 = vst[:].rearrange("p (h s) -> p h s", s=HD + 1)
                nc.vector.tensor_copy(
                    vst3[:, :, 0:HD],
                    ps[:].rearrange("p (h s) -> p h s", s=HD))
                nc.vector.tensor_copy(
                    vst3[:, :, HD:HD + 1],
                    onesr[:].rearrange("p (h o) -> p h o", o=1))
                nc.sync.dma_start(Vp[t0:t0 + 128, :], vst[:])
    return qts


def _phase_b(nc, tc, ot_pool, qts, mb_sb, ones64, KTd, Vp):
    from contextlib import ExitStack

    ots = []
    with ExitStack() as s:
        kh_pool = s.enter_context(tc.tile_pool(name="kh", bufs=2))
        vh_pool = s.enter_context(tc.tile_pool(name="vh", bufs=2))
        p_pool = s.enter_context(tc.tile_pool(name="p", bufs=3))
        nrm_pool = s.enter_context(tc.tile_pool(name="nrm", bufs=2))
        sp_pool = s.enter_context(
            tc.tile_pool(name="sp", bufs=2, space="PSUM"))
        op_pool = s.enter_context(
            tc.tile_pool(name="op", bufs=1, space="PSUM"))
        bc_pool = s.enter_context(
            tc.tile_pool(name="bc", bufs=1, space="PSUM"))

        for h in range(H):
            kh = kh_pool.tile([64, T], MMDT, tag="kh")
            nc.sync.dma_start(kh[:], KTd[h * 64:(h + 1) * 64, :])
            vh = vh_pool.tile([128, KT * (HD + 1)], MMDT, tag="vh")
            nc.sync.dma_start(
                vh[:].rearrange("p (kt s) -> p kt s", s=HD + 1),
                Vp.rearrange("(kt p) (h s) -> p kt h s", p=128,
                             s=HD + 1)[:, :, h, :])

            ops = [op_pool.tile([65, 512], F32, tag=f"op{c}", name=f"op{c}")
                   for c in range(QC)]
            for kt in range(KT):
                sp = sp_pool.tile([128, QC * 512], F32, tag="sp")
                for c in range(QC):
                    nc.tensor.matmul(
                        sp[:, c * 512:(c + 1) * 512],
                        kh[:, kt * 128:(kt + 1) * 128],
                        qts[h][:, c * 512:(c + 1) * 512],
                        start=True, stop=True, skip_group_check=True)
                p = p_pool.tile([128, QC * 512], MMDT, tag="p")
                nc.scalar.activation(p[:], sp[:], AF.Exp,
                                     bias=mb_sb[:, kt:kt + 1], scale=0.125)
                for c in range(QC):
                    nc.tensor.matmul(
                        ops[c][:],
                        vh[:, kt * (HD + 1):(kt + 1) * (HD + 1)],
                        p[:, c * 512:(c + 1) * 512],
                        start=(kt == 0), stop=(kt == KT - 1))

            ot = ot_pool.tile([64, QS], MMDT, tag=f"ot{h}")
            for c in range(QC):
                recip = nrm_pool.tile([1, 512], F32, tag="recip")
                nc.vector.reciprocal(recip[:], ops[c][64:65, :])
                bc = bc_pool.tile([64, 512], F32, tag="bc")
                nc.tensor.matmul(bc[:], ones64[:], recip[:],
                                 start=True, stop=True)
                bc_sb = nrm_pool.tile([64, 512], F32, tag="bc_sb")
                nc.vector.tensor_copy(bc_sb[:], bc[:])
                nc.vector.tensor_mul(ot[:, c * 512:(c + 1) * 512],
                                     ops[c][0:64, :], bc_sb[:])
            ots.append(ot)
    return ots


def _phase_c(nc, tc, ots, wpT, bp_sb, outT):
    from contextlib import ExitStack

    with ExitStack() as s:
        wp_pool = s.enter_context(tc.tile_pool(name="wp", bufs=1))
        ost_pool = s.enter_context(tc.tile_pool(name="ost", bufs=3))
        pps_pool = s.enter_context(
            tc.tile_pool(name="pps", bufs=2, space="PSUM"))

        wp_sb = []
        for h in range(H):
            wp = wp_pool.tile([64, D], MMDT, tag=f"wp{h}")
            nc.sync.dma_start(wp[:], wpT[h * 64:(h + 1) * 64, :])
            wp_sb.append(wp)

        for m in range(DT):
            for c in range(QC):
                ps = pps_pool.tile([128, 512], F32, tag="pps")
                for h in range(H):
                    nc.tensor.matmul(
                        ps[:], wp_sb[h][:, m * 128:(m + 1) * 128],
                        ots[h][:, c * 512:(c + 1) * 512],
                        start=(h == 0), stop=(h == H - 1))
                ost = ost_pool.tile([128, 512], F32, tag="ost")
                nc.vector.tensor_scalar_add(ost[:], ps[:], bp_sb[:, m:m + 1])
                nc.sync.dma_start(
                    outT[m * 128:(m + 1) * 128, c * 512:(c + 1) * 512],
                    ost[:])


# ---------------------------------------------------------------- host side

@functools.lru_cache(maxsize=None)
def _get_runner(reps: int = 1):
    import jax
    from jax.sharding import Mesh, PartitionSpec
    from jax.experimental.shard_map import shard_map

    nc = build_program(reps)
    install_neuronx_cc_hook()
    partition_name = (nc.partition_id_tensor.name
                      if nc.partition_id_tensor else None)
    in_names, out_names, out_avals, out_shapes = [], [], [], []
    for alloc in nc.m.functions[0].allocations:
        if not isinstance(alloc, mybir.MemoryLocationSet):
            continue
        name = alloc.memorylocations[0].name
        if alloc.kind == "ExternalInput":
            if name != partition_name:
                in_names.append(name)
        elif alloc.kind == "ExternalOutput":
            out_names.append(name)
            shape = tuple(alloc.tensor_shape)
            dtype = mybir.dt.np(alloc.dtype)
            out_avals.append(jax.core.ShapedArray(shape, dtype))
            out_shapes.append((shape, dtype))
    n_params = len(in_names)
    n_outs = len(out_avals)
    all_in_names = list(in_names) + list(out_names)
    if partition_name is not None:
        all_in_names.append(partition_name)
    donate = tuple(range(n_params, n_params + n_outs))

    def _bodyf(*args):
        operands = list(args)
        if partition_name is not None:
            operands.append(partition_id_tensor())
        outs = _bass_exec_p.bind(
            *operands,
            out_avals=tuple(out_avals),
            in_names=tuple(all_in_names),
            out_names=tuple(out_names),
            lowering_input_output_aliases=(),
            sim_require_finite=True,
            sim_require_nnan=True,
            nc=nc,
        )
        return tuple(outs)

    devices = jax.devices()[:N_CORES]
    mesh = Mesh(np.asarray(devices), ("core",))
    in_specs = (PartitionSpec("core"),) * (n_params + n_outs)
    out_specs = (PartitionSpec("core"),) * len(out_names)
    sharded = jax.jit(
        shard_map(_bodyf, mesh=mesh, in_specs=in_specs, out_specs=out_specs,
                  check_rep=False),
        donate_argnums=donate, keep_unused=True,
    )

    def run(in_maps):
        import jax as _jax
        per_core = [[np.asarray(m[n]) for n in in_names] for m in in_maps]
        concat_in = [np.concatenate([per_core[c][i] for c in range(N_CORES)],
                                    axis=0) for i in range(n_params)]
        concat_zeros = [np.zeros((N_CORES * s[0], *s[1:]), dt)
                        for (s, dt) in out_shapes]
        out_arrs = sharded(*concat_in, *concat_zeros)
        _jax.block_until_ready(out_arrs)
        return [
            {name: np.asarray(out_arrs[i]).reshape(
                N_CORES, *out_shapes[i][0])[c]
             for i, name in enumerate(out_names)}
            for c in range(N_CORES)
        ]

    return run


def make_in_maps(x, mask, w_qkv, w_proj, b_proj):
    import ml_dtypes
    mm_np = ml_dtypes.bfloat16 if USE_BF16 else np.float32
    x = np.asarray(x, np.float32)
    mask = np.asarray(mask)
    w_qkv = np.asarray(w_qkv, np.float32)
    w_proj = np.asarray(w_proj, np.float32)
    b_proj = np.asarray(b_proj, np.float32)

    wqT = np.ascontiguousarray(w_qkv[0:D].T).astype(mm_np)
    wkT = np.ascontiguousarray(w_qkv[D:2 * D].T).astype(mm_np)
    wvT = np.ascontiguousarray(w_qkv[2 * D:3 * D].T).astype(mm_np)
    wpT = np.ascontiguousarray(w_proj.T).astype(mm_np)
    bp = np.ascontiguousarray(b_proj.reshape(DT, 128).T)
    onesc = np.ones((128, H), mm_np)

    xTs = [np.ascontiguousarray(x[b].T).astype(mm_np) for b in range(B)]
    mbs = [np.ascontiguousarray(
        np.where(mask[b], np.float32(-NB), np.float32(0.0))
        .astype(np.float32).reshape(KT, 128).T) for b in range(B)]

    in_maps = []
    for c in range(N_CORES):
        b, qi = divmod(c, CORES_PER_B)
        q0 = qi * QS
        in_maps.append({
            "xT": xTs[b],
            "xTq": np.ascontiguousarray(xTs[b][:, q0:q0 + QS]),
            "wqT": wqT, "wkT": wkT, "wvT": wvT, "wpT": wpT,
            "bp": bp, "mb": mbs[b], "onesc": onesc,
        })
    return in_maps


def assemble_output(results):
    out = np.empty((B, T, D), np.float32)
    for c in range(N_CORES):
        b, qi = divmod(c, CORES_PER_B)
        q0 = qi * QS
        out[b, q0:q0 + QS, :] = results[c]["outT"].T
    return out


def kernel(x, mask, w_qkv, w_proj, b_proj):
    run = _get_runner(1)
    in_maps = make_in_maps(x, mask, w_qkv, w_proj, b_proj)
    results = run(in_maps)
    return assemble_output(results)
